# revision 1
# baseline (speedup 1.0000x reference)
"""GCN classifier (2x GCNConv + add-pool + MLP) on 8 trn2 NeuronCores via Bass/Tile.

Strategy (dst-stationary node sharding):
  - Nodes are split into 8 contiguous shards; core k owns all in-edges of its
    shard (self-loops included as explicit edges with coefficient dinv^2).
  - Per-edge coefficient c = dinv[src]*ew*dinv[dst] is folded into a weighted
    one-hot "selection" matrix built on DVE (one tensor_scalar per 128-edge
    chunk); aggregation is a bf16 matmul accumulating into PSUM per 128-dst
    block: psumT[f, d] += gathered_src_rows.T @ sel.
  - Source rows are fetched with dma_gather (int16 local indices, 256B rows)
    from a replicated DRAM table: the padded x table for layer 1, the
    AllGathered bf16 h1 table for layer 2.
  - Pooling: per block one full-width [128,512] one-hot (absolute graph ids)
    matmul accumulated in a dedicated PSUM bank; only the pooled [128,512]
    tensor is AllReduced before the (replicated) MLP head.
"""

import os
import sys
import types

sys.path.insert(0, "/opt/trn_rl_repo")

import numpy as np
import ml_dtypes

import concourse.mybir as mybir
import concourse.tile as tile
from concourse import bacc
from concourse.bass_utils import run_bass_kernel_spmd
from concourse.masks import make_identity

P = 128
N_CORES = 8
IN_DIM = 64
HID = 128
OUT_DIM = 10
N_GRAPHS = 512
BLOCKS_PER_BATCH = 4       # dst blocks resident in one PSUM bank
N_GROUPS = 2               # src index groups (int16 range / overlap granularity)
BF = ml_dtypes.bfloat16

_TRACE = os.environ.get("BASS_GCN_TRACE", "") == "1"
_STOP = os.environ.get("BASS_GCN_STOP", "")  # "l1"|"ag"|"l2"|"" bisection


# --------------------------------------------------------------------------
# NTFF profile hook shim (antenv.axon_hooks is absent in this image)
# --------------------------------------------------------------------------
def _install_profhook():
    if "antenv.axon_hooks" in sys.modules:
        return
    so_path = "/opt/axon/libaxon_pjrt.so"
    if not os.path.exists(so_path):
        return
    sys.path.insert(0, "/root/.axon_site")
    try:
        from trn_agent_boot.trn_boot import _ntff_profile_via_ctypes
    except Exception:
        return
    holder = {"hook": None}
    mod = types.ModuleType("antenv.axon_hooks")
    mod.set_axon_ntff_profile_hook = lambda h: holder.__setitem__("hook", h)
    mod.get_axon_ntff_profile_hook = lambda: holder["hook"]
    sys.modules["antenv.axon_hooks"] = mod
    import antenv

    antenv.axon_hooks = mod
    mod.set_axon_ntff_profile_hook(_ntff_profile_via_ctypes(so_path))


# --------------------------------------------------------------------------
# Host-side preprocessing: shard + sort + pack edge metadata
# --------------------------------------------------------------------------
class Plan:
    """Static (core-independent) program structure + per-core packed arrays."""


def _build_plan(x, edge_index, batch, edge_attr):
    N = x.shape[0]
    assert N % N_CORES == 0
    SH = N // N_CORES                      # nodes per core shard
    n_blocks = (SH + P - 1) // P           # dst blocks per core
    n_batches = (n_blocks + BLOCKS_PER_BATCH - 1) // BLOCKS_PER_BATCH
    grp_size = (N + N_GROUPS - 1) // N_GROUPS
    assert grp_size <= 32768

    src = edge_index[0].astype(np.int64)
    dst = edge_index[1].astype(np.int64)
    ew = edge_attr.astype(np.float32)

    # symmetric GCN normalization with self-loops (matches reference)
    deg = np.bincount(dst, weights=ew, minlength=N).astype(np.float32) + 1.0
    dinv = 1.0 / np.sqrt(deg)

    allsrc = np.concatenate([src, np.arange(N, dtype=np.int64)])
    alldst = np.concatenate([dst, np.arange(N, dtype=np.int64)])
    allc = np.concatenate([dinv[src] * ew * dinv[dst], dinv * dinv]).astype(np.float32)

    core = alldst // SH
    dloc = alldst - core * SH              # 0..SH-1
    blk = dloc // P                        # 0..n_blocks-1
    bat = blk // BLOCKS_PER_BATCH
    grp = allsrc // grp_size

    # order: core, batch, group, block, src
    order = np.lexsort((allsrc, blk, grp, bat, core))
    c_src = allsrc[order]
    c_blk = blk[order]
    c_bat = bat[order]
    c_grp = grp[order]
    c_core = core[order]
    c_dl = (dloc[order] - c_blk * P).astype(np.float32)  # 0..127 within block
    c_c = allc[order]
    c_srcloc = (c_src - c_grp * grp_size).astype(np.int64)

    # per-(core,batch,group,block) counts
    key = ((c_core * n_batches + c_bat) * N_GROUPS + c_grp) * n_blocks + c_blk
    counts = np.bincount(key, minlength=N_CORES * n_batches * N_GROUPS * n_blocks)
    counts = counts.reshape(N_CORES, n_batches, N_GROUPS, n_blocks)
    # unified chunk counts (max over cores)
    nch = np.ceil(counts / P).astype(np.int64).max(axis=0)  # [n_batches, N_GROUPS, n_blocks]

    plan = Plan()
    plan.N, plan.SH = N, SH
    plan.n_blocks, plan.n_batches = n_blocks, n_batches
    plan.grp_size = grp_size
    plan.nch = nch

    # chunk schedule, BLOCK-major within a batch (an accumulation group's
    # start=True clears has_written for the whole PSUM bank, so different
    # blocks sharing a bank must not interleave their groups).
    # Each entry: (g, ci_within_call_g, block, start, stop)
    sched = []
    for b in range(n_batches):
        blocks_here = list(range(b * BLOCKS_PER_BATCH,
                                 min((b + 1) * BLOCKS_PER_BATCH, n_blocks)))
        ci = [0] * N_GROUPS
        chunks = []
        for j in blocks_here:
            tot = int(nch[b, :, j].sum())
            seen = 0
            for g in range(N_GROUPS):
                for _ in range(int(nch[b, g, j])):
                    seen += 1
                    chunks.append((g, ci[g], j, seen == 1, seen == tot))
                    ci[g] += 1
        sched.append(chunks)
    plan.sched = sched
    plan.call_nch = [[int(plan.nch[b, g].sum()) for g in range(N_GROUPS)]
                     for b in range(n_batches)]

    # pack per-core arrays (stream order: batch -> group -> block -> chunks)
    flat_off = np.zeros(counts.size + 1, np.int64)
    np.cumsum(counts.ravel(), out=flat_off[1:])
    starts = flat_off[:-1].reshape(counts.shape)

    idx_parts, dl_parts, cv_parts = [], [], []
    for k in range(N_CORES):
        k_idx, k_dl, k_cv = [], [], []
        for b in range(n_batches):
            for g in range(N_GROUPS):
                if plan.call_nch[b][g] == 0:
                    continue
                call_idx, call_dl, call_cv = [], [], []
                for j in range(n_blocks):
                    n_pad = int(nch[b, g, j]) * P
                    if n_pad == 0:
                        continue
                    o = starts[k, b, g, j]
                    cnt = counts[k, b, g, j]
                    si = np.zeros(n_pad, np.int16)
                    dli = np.zeros(n_pad, np.float32)
                    cvi = np.zeros(n_pad, np.float32)
                    si[:cnt] = c_srcloc[o:o + cnt]
                    dli[:cnt] = c_dl[o:o + cnt]
                    cvi[:cnt] = c_c[o:o + cnt]
                    call_idx.append(si)
                    call_dl.append(dli)
                    call_cv.append(cvi)
                ci_arr = np.concatenate(call_idx)
                nidx = len(ci_arr)
                # wrapped-16 idx layout, replicated to 8 groups of 16 partitions
                wrapped = np.tile(ci_arr.reshape(nidx // 16, 16).T, (8, 1))
                k_idx.append(wrapped.ravel())
                k_dl.append(np.concatenate(call_dl).reshape(-1, P).T.ravel())
                k_cv.append(np.concatenate(call_cv).reshape(-1, P).T.ravel())
        idx_parts.append(np.concatenate(k_idx).astype(np.int16))
        dl_parts.append(np.concatenate(k_dl).astype(np.float32))
        cv_parts.append(np.concatenate(k_cv).astype(np.float32))
    plan.idx = idx_parts      # per core flat [128 * total_idx/16]
    plan.dl = dl_parts
    plan.cv = cv_parts

    # pooling metadata: absolute graph id per node (f32), -1 for pad rows
    bl_cols = np.full((N_CORES, n_blocks, P), -1.0, np.float32)
    for k in range(N_CORES):
        for j in range(n_blocks):
            lo = k * SH + j * P
            hi = min(lo + P, (k + 1) * SH)
            if lo < hi:
                bl_cols[k, j, :hi - lo] = batch[lo:hi].astype(np.float32)
    assert bl_cols.max() < N_GRAPHS
    plan.bl_cols = bl_cols
    return plan


# --------------------------------------------------------------------------
# Device kernel build
# --------------------------------------------------------------------------
def _build_nc(plan):
    N, SH = plan.N, plan.SH
    n_blocks, n_batches = plan.n_blocks, plan.n_batches
    SH_PAD = n_blocks * P
    f32, bf16, i16 = mybir.dt.float32, mybir.dt.bfloat16, mybir.dt.int16
    AF = mybir.ActivationFunctionType
    OP = mybir.AluOpType

    nc = bacc.Bacc(None, target_bir_lowering=False, num_devices=N_CORES,
                   num_swdge_queues=2)

    n_idx16 = plan.idx[0].size // P      # idx dram columns
    n_ch_tot = plan.dl[0].size // P      # total chunks per layer stream

    xt = nc.dram_tensor("xt", [N, P], bf16, kind="ExternalInput")
    idx_d = nc.dram_tensor("idxd", [P * n_idx16], i16, kind="ExternalInput")
    dl_d = nc.dram_tensor("dld", [P * n_ch_tot], f32, kind="ExternalInput")
    cv_d = nc.dram_tensor("cvd", [P * n_ch_tot], f32, kind="ExternalInput")
    w1_d = nc.dram_tensor("w1", [IN_DIM, HID], f32, kind="ExternalInput")
    w2_d = nc.dram_tensor("w2", [HID, HID], f32, kind="ExternalInput")
    wm1_d = nc.dram_tensor("wm1", [HID, HID], f32, kind="ExternalInput")
    wm2_d = nc.dram_tensor("wm2", [HID, OUT_DIM], f32, kind="ExternalInput")
    b1_d = nc.dram_tensor("b1", [HID, 1], f32, kind="ExternalInput")
    b2_d = nc.dram_tensor("b2", [HID, 1], f32, kind="ExternalInput")
    bm1_d = nc.dram_tensor("bm1", [HID, 1], f32, kind="ExternalInput")
    bm2_d = nc.dram_tensor("bm2", [OUT_DIM, 1], f32, kind="ExternalInput")
    bl_d = nc.dram_tensor("bl", [P, n_blocks], f32, kind="ExternalInput")
    out_d = nc.dram_tensor("out", [OUT_DIM, N_GRAPHS], f32, kind="ExternalOutput")

    with tile.TileContext(nc) as tc:
        with (
            tc.tile_pool(name="const", bufs=1) as cpool,
            tc.tile_pool(name="meta", bufs=5) as mpool,
            tc.tile_pool(name="gat", bufs=8) as gpool,
            tc.tile_pool(name="work", bufs=2) as wpool,
            tc.tile_pool(name="ps", bufs=2, space="PSUM") as ppool,
            tc.tile_pool(name="dram", bufs=1, space="DRAM") as dpool,
        ):
            # ---- constants ----
            iota_f = cpool.tile([P, P], f32)
            nc.gpsimd.iota(iota_f[:], pattern=[[1, P]], base=0, channel_multiplier=0,
                           allow_small_or_imprecise_dtypes=True)
            iota_fb = cpool.tile([P, P], bf16)
            nc.vector.tensor_copy(iota_fb[:], iota_f[:])
            iota_g = cpool.tile([P, N_GRAPHS], f32)
            nc.gpsimd.iota(iota_g[:], pattern=[[1, N_GRAPHS]], base=0,
                           channel_multiplier=0,
                           allow_small_or_imprecise_dtypes=True)
            ident = cpool.tile([P, P], bf16)
            make_identity(nc, ident[:])

            w1b = cpool.tile([IN_DIM, HID], bf16)
            nc.gpsimd.dma_start(w1b[:], w1_d[:])      # SWDGE cast f32->bf16
            w2b = cpool.tile([HID, HID], bf16)
            nc.gpsimd.dma_start(w2b[:], w2_d[:])
            wm1b = cpool.tile([HID, HID], bf16)
            nc.gpsimd.dma_start(wm1b[:], wm1_d[:])
            wm2b = cpool.tile([HID, OUT_DIM], bf16)
            nc.gpsimd.dma_start(wm2b[:], wm2_d[:])
            b1s = cpool.tile([HID, 1], f32)
            nc.sync.dma_start(b1s[:], b1_d[:])
            b2s = cpool.tile([HID, 1], f32)
            nc.sync.dma_start(b2s[:], b2_d[:])
            bm1s = cpool.tile([HID, 1], f32)
            nc.sync.dma_start(bm1s[:], bm1_d[:])
            bm2s = cpool.tile([OUT_DIM, 1], f32)
            nc.sync.dma_start(bm2s[:], bm2_d[:])
            bls = cpool.tile([P, n_blocks], f32)
            nc.sync.dma_start(bls[:], bl_d[:])

            h1_shard = dpool.tile([SH_PAD, HID], bf16)
            h1_table = dpool.tile([N, HID], bf16, addr_space="Shared")
            cc_in = dpool.tile([P, N_GRAPHS], f32)
            cc_out = dpool.tile([P, N_GRAPHS], f32, addr_space="Shared")

            pool_ps = ppool.tile([HID, N_GRAPHS], f32, tag="pw", bufs=1,
                                 name="pool_ps")

            def layer(lnum, table, feat_dim):
                io = {"idx": 0, "ch": 0}
                for b in range(n_batches):
                    agg = ppool.tile([feat_dim, P * BLOCKS_PER_BATCH], f32,
                                     tag="agg", name=f"agg{lnum}_{b}")
                    gts, dls, cvs = {}, {}, {}
                    for g in range(N_GROUPS):
                        ncall = plan.call_nch[b][g]
                        if ncall == 0:
                            continue
                        nidx = ncall * P
                        s16 = nidx // 16
                        idx_t = mpool.tile([P, s16], i16, tag="idx",
                                           name=f"idx{lnum}_{b}_{g}")
                        nc.sync.dma_start(
                            idx_t[:],
                            idx_d[P * io["idx"]: P * (io["idx"] + s16)]
                            .rearrange("(p c) -> p c", p=P))
                        dl_t = mpool.tile([P, ncall], f32, tag="dl",
                                          name=f"dl{lnum}_{b}_{g}")
                        nc.sync.dma_start(
                            dl_t[:],
                            dl_d[P * io["ch"]: P * (io["ch"] + ncall)]
                            .rearrange("(p c) -> p c", p=P))
                        cv_t = mpool.tile([P, ncall], f32, tag="cv",
                                          name=f"cv{lnum}_{b}_{g}")
                        nc.sync.dma_start(
                            cv_t[:],
                            cv_d[P * io["ch"]: P * (io["ch"] + ncall)]
                            .rearrange("(p c) -> p c", p=P))
                        tab_ap = table[g * plan.grp_size:
                                       min((g + 1) * plan.grp_size, N), :]
                        nsplit = 2 if ncall >= 8 else 1
                        bnds = [ncall * k // nsplit for k in range(nsplit + 1)]
                        gouts, cum = [], []
                        for si in range(nsplit):
                            c0, c1 = bnds[si], bnds[si + 1]
                            go = gpool.tile([P, c1 - c0, P], bf16, tag="g",
                                            name=f"g{si}_{lnum}_{b}_{g}")
                            nc.gpsimd.dma_gather(
                                out_ap=go[:],
                                in_ap=tab_ap,
                                idxs_ap=idx_t[:, c0 * 8:c1 * 8],
                                num_idxs=(c1 - c0) * P,
                                num_idxs_reg=(c1 - c0) * P,
                                elem_size=P,
                                single_packet=False,
                                queue_num=(b * N_GROUPS * 2 + g * 2 + si) % 2,
                            )
                            gouts.append(go)
                            cum.append(c0)
                        gts[g], dls[g], cvs[g] = (gouts, cum, bnds), dl_t, cv_t
                        io["idx"] += s16
                        io["ch"] += ncall
                    for (g, ci, j, st, sp) in plan.sched[b]:
                        jj = j - b * BLOCKS_PER_BATCH
                        sel = wpool.tile([P, P], bf16, tag="sel",
                                         name=f"sel{lnum}_{b}_{g}_{ci}")
                        nc.vector.tensor_scalar(
                            out=sel[:], in0=iota_fb[:],
                            scalar1=dls[g][:, ci:ci + 1],
                            scalar2=cvs[g][:, ci:ci + 1],
                            op0=OP.is_equal, op1=OP.mult)
                        gouts, cum, bnds = gts[g]
                        pi = 0
                        while pi + 1 < len(bnds) - 1 and ci >= bnds[pi + 1]:
                            pi += 1
                        gsrc = gouts[pi][:, ci - cum[pi], :feat_dim]
                        nc.tensor.matmul(
                            out=agg[:, jj * P:(jj + 1) * P],
                            lhsT=gsrc,
                            rhs=sel[:],
                            start=st, stop=sp)
                    # flush the batch
                    for j in range(b * BLOCKS_PER_BATCH,
                                   min((b + 1) * BLOCKS_PER_BATCH, n_blocks)):
                        jj = j - b * BLOCKS_PER_BATCH
                        o_t = wpool.tile([feat_dim, P], bf16, tag="o",
                                         name=f"o{lnum}_{b}_{j}")
                        nc.any.tensor_copy(o_t[:], agg[:, jj * P:(jj + 1) * P])
                        zp = ppool.tile([HID, P], f32, tag="ztr",
                                        name=f"zp{lnum}_{b}_{j}")
                        wmat = w1b if lnum == 1 else w2b
                        bvec = b1s if lnum == 1 else b2s
                        nc.tensor.matmul(out=zp[:], lhsT=wmat[:], rhs=o_t[:],
                                         start=True, stop=True)
                        zs = wpool.tile([HID, P], bf16, tag="zs",
                                        name=f"zs{lnum}_{b}_{j}")
                        nc.scalar.activation(zs[:], zp[:], AF.Relu, bias=bvec[:, :1])
                        trp = ppool.tile([P, HID], bf16, tag="tr",
                                         name=f"trp{lnum}_{b}_{j}")
                        nc.tensor.transpose(out=trp[:], in_=zs[:], identity=ident[:])
                        hb = wpool.tile([P, HID], bf16, tag="hb",
                                        name=f"hb{lnum}_{b}_{j}")
                        nc.any.tensor_copy(hb[:], trp[:])
                        if lnum == 1:
                            nc.sync.dma_start(h1_shard[j * P:(j + 1) * P, :], hb[:])
                        else:
                            selB = wpool.tile([P, N_GRAPHS], bf16, tag="selB",
                                              name=f"selB{b}_{j}")
                            nc.vector.tensor_scalar(
                                out=selB[:], in0=iota_g[:],
                                scalar1=bls[:, j:j + 1], scalar2=None,
                                op0=OP.is_equal)
                            nc.tensor.matmul(out=pool_ps[:], lhsT=hb[:],
                                             rhs=selB[:],
                                             start=(j == 0),
                                             stop=(j == n_blocks - 1))

            def early_out():
                outf = cpool.tile([OUT_DIM, N_GRAPHS], f32, name="outf_e")
                nc.vector.memset(outf[:], 0.0)
                nc.sync.dma_start(out_d[:], outf[:])

            # ---- layer 1 (aggregate raw x in 64-dim space) ----
            layer(1, xt, IN_DIM)
            done = _STOP == "l1"

            # ---- AllGather h1 ----
            if not done:
                nc.gpsimd.collective_compute(
                    "AllGather", mybir.AluOpType.bypass,
                    replica_groups=[list(range(N_CORES))],
                    ins=[h1_shard[0:SH, :].opt()],
                    outs=[h1_table[:].opt()],
                )
                done = _STOP == "ag"

            # ---- layer 2 ----
            if not done:
                layer(2, h1_table, HID)
                done = _STOP == "l2"

            # ---- pooled AllReduce + MLP head ----
            if done:
                early_out()
                do_tail = False
            else:
                do_tail = True
            if do_tail:
                pooledT = cpool.tile([P, N_GRAPHS], f32)
                nc.any.tensor_copy(pooledT[:], pool_ps[:])
                nc.sync.dma_start(cc_in[:], pooledT[:])
                nc.gpsimd.collective_compute(
                    "AllReduce", mybir.AluOpType.add,
                    replica_groups=[list(range(N_CORES))],
                    ins=[cc_in[:].opt()],
                    outs=[cc_out[:].opt()],
                )
                pall = cpool.tile([P, N_GRAPHS], f32)
                nc.sync.dma_start(pall[:], cc_out[:])
                pbf = cpool.tile([P, N_GRAPHS], bf16)
                nc.vector.tensor_copy(pbf[:], pall[:])
                m1p = ppool.tile([HID, N_GRAPHS], f32, tag="agg", name="m1p")
                nc.tensor.matmul(out=m1p[:], lhsT=wm1b[:], rhs=pbf[:],
                                 start=True, stop=True)
                m1s = cpool.tile([HID, N_GRAPHS], bf16)
                nc.scalar.activation(m1s[:], m1p[:], AF.Relu, bias=bm1s[:, :1])
                m2p = ppool.tile([OUT_DIM, N_GRAPHS], f32, tag="ztr", name="m2p")
                nc.tensor.matmul(out=m2p[:], lhsT=wm2b[:], rhs=m1s[:],
                                 start=True, stop=True)
                outf = cpool.tile([OUT_DIM, N_GRAPHS], f32)
                nc.vector.tensor_scalar(out=outf[:], in0=m2p[:],
                                        scalar1=bm2s[:, :1], scalar2=None,
                                        op0=OP.add)
                nc.sync.dma_start(out_d[:], outf[:])

    nc.finalize()
    return nc


# --------------------------------------------------------------------------
# Public entry point
# --------------------------------------------------------------------------
def kernel(x, edge_index, batch, edge_attr, W1, b1, W2, b2, Wm1, bm1, Wm2, bm2):
    x = np.asarray(x, np.float32)
    edge_index = np.asarray(edge_index, np.int64)
    batch_np = np.asarray(batch, np.int64)
    edge_attr = np.asarray(edge_attr, np.float32)
    N = x.shape[0]

    _install_profhook()
    plan = _build_plan(x, edge_index, batch_np, edge_attr)

    # padded bf16 x table [N, 128] (first 64 cols = x)
    xt = np.zeros((N, P), BF)
    xt[:, :IN_DIM] = x.astype(BF)

    in_maps = []
    for k in range(N_CORES):
        in_maps.append({
            "xt": xt,
            "idxd": plan.idx[k],
            "dld": plan.dl[k],
            "cvd": plan.cv[k],
            "w1": np.asarray(W1, np.float32),
            "w2": np.asarray(W2, np.float32),
            "wm1": np.asarray(Wm1, np.float32),
            "wm2": np.asarray(Wm2, np.float32),
            "b1": np.asarray(b1, np.float32).reshape(HID, 1),
            "b2": np.asarray(b2, np.float32).reshape(HID, 1),
            "bm1": np.asarray(bm1, np.float32).reshape(HID, 1),
            "bm2": np.asarray(bm2, np.float32).reshape(OUT_DIM, 1),
            "bl": plan.bl_cols[k].T.copy(),     # [128, n_blocks]
        })

    nc = _build_nc(plan)
    res = run_bass_kernel_spmd(nc, in_maps, list(range(N_CORES)), trace=_TRACE)
    if _TRACE:
        kernel.last_exec_time_ns = res.exec_time_ns
        kernel.last_results = res
    out = np.asarray(res.results[0]["out"], np.float32)  # [10, 512]
    return np.ascontiguousarray(out.T)



# revision 8
# speedup vs baseline: 1.1174x; 1.1174x over previous
"""GCN classifier (2x GCNConv + add-pool + MLP) on 8 trn2 NeuronCores via Bass/Tile.

Strategy (dst-stationary node sharding), v2:
  - Nodes split into 8 contiguous shards; core k owns all in-edges of its shard.
  - Per-edge coefficient c = dinv[src]*ew*dinv[dst] folded into a weighted
    one-hot selection matrix built on DVE; aggregation is a bf16 matmul
    accumulating into PSUM per 128-dst block.
  - Source rows fetched with dma_gather (int16 local indices, 256B rows).
    The gather descriptor-generation rate (~4.7ns/idx, SWDGE Q7-serial) is the
    kernel's critical path, so v2:
      * shuffles edge order inside each (batch,group,block) cell (sorted idx
        streams are ~20% slower through the gather ucode),
      * drops self-loops from the gather stream (their contribution
        dinv^2 * x[d] is added by one per-block matmul from the core's own
        shard, resident in SBUF),
      * merges each (batch,group)'s gather into one up-to-9216-idx call.
  - Pooling: per block one [128,512] one-hot matmul into a dedicated PSUM
    bank; only the pooled [128,512] tensor is AllReduced before the MLP head.
"""

import os
import sys
import types

sys.path.insert(0, "/opt/trn_rl_repo")

import numpy as np
import ml_dtypes

import concourse.mybir as mybir
import concourse.tile as tile
from concourse import bacc
from concourse.bass_utils import run_bass_kernel_spmd
from concourse.masks import make_identity

P = 128
N_CORES = 8
IN_DIM = 64
HID = 128
OUT_DIM = 10
N_GRAPHS = 512
BLOCKS_PER_BATCH = 4       # dst blocks resident in one PSUM bank
N_GROUPS = 2               # src index groups (int16 range)
BF = ml_dtypes.bfloat16

_TRACE = os.environ.get("BASS_GCN_TRACE", "") == "1"
_STOP = os.environ.get("BASS_GCN_STOP", "")  # "l1"|"ag"|"l2"|"" bisection


# --------------------------------------------------------------------------
# NTFF profile hook shim (antenv.axon_hooks is absent in this image)
# --------------------------------------------------------------------------
def _install_profhook():
    if "antenv.axon_hooks" in sys.modules:
        return
    so_path = "/opt/axon/libaxon_pjrt.so"
    if not os.path.exists(so_path):
        return
    sys.path.insert(0, "/root/.axon_site")
    try:
        from trn_agent_boot.trn_boot import _ntff_profile_via_ctypes
    except Exception:
        return
    holder = {"hook": None}
    mod = types.ModuleType("antenv.axon_hooks")
    mod.set_axon_ntff_profile_hook = lambda h: holder.__setitem__("hook", h)
    mod.get_axon_ntff_profile_hook = lambda: holder["hook"]
    sys.modules["antenv.axon_hooks"] = mod
    import antenv

    antenv.axon_hooks = mod
    mod.set_axon_ntff_profile_hook(_ntff_profile_via_ctypes(so_path))


# --------------------------------------------------------------------------
# Host-side preprocessing: shard + sort + pack edge metadata
# --------------------------------------------------------------------------
class Plan:
    """Static (core-independent) program structure + per-core packed arrays."""


def _build_plan(x, edge_index, batch, edge_attr):
    N = x.shape[0]
    assert N % N_CORES == 0
    SH = N // N_CORES                      # nodes per core shard
    n_blocks = (SH + P - 1) // P           # dst blocks per core
    n_batches = (n_blocks + BLOCKS_PER_BATCH - 1) // BLOCKS_PER_BATCH
    grp_size = (N + N_GROUPS - 1) // N_GROUPS
    assert grp_size <= 32768

    src = edge_index[0].astype(np.int64)
    dst = edge_index[1].astype(np.int64)
    ew = edge_attr.astype(np.float32)

    # symmetric GCN normalization with self-loops (matches reference); the
    # self-loop term itself is applied on-device from the core's own shard.
    deg = np.bincount(dst, weights=ew, minlength=N).astype(np.float32) + 1.0
    dinv = 1.0 / np.sqrt(deg)

    allc = (dinv[src] * ew * dinv[dst]).astype(np.float32)

    core = dst // SH
    dloc = dst - core * SH                 # 0..SH-1
    blk = dloc // P                        # 0..n_blocks-1
    bat = blk // BLOCKS_PER_BATCH
    grp = src // grp_size

    order = np.lexsort((src, blk, grp, bat, core))
    c_src = src[order]
    c_blk = blk[order]
    c_bat = bat[order]
    c_grp = grp[order]
    c_core = core[order]
    c_dl = (dloc[order] - c_blk * P).astype(np.float32)  # 0..127 within block
    c_c = allc[order]
    c_srcloc = (c_src - c_grp * grp_size).astype(np.int64)

    key = ((c_core * n_batches + c_bat) * N_GROUPS + c_grp) * n_blocks + c_blk
    counts = np.bincount(key, minlength=N_CORES * n_batches * N_GROUPS * n_blocks)
    counts = counts.reshape(N_CORES, n_batches, N_GROUPS, n_blocks)
    nch = np.ceil(counts / P).astype(np.int64).max(axis=0)  # [n_batches,G,n_blocks]

    plan = Plan()
    plan.N, plan.SH = N, SH
    plan.n_blocks, plan.n_batches = n_blocks, n_batches
    plan.grp_size = grp_size
    plan.nch = nch

    # chunk schedule, BLOCK-major within a batch. Entry kinds:
    #   ('self', j, start, stop)  — per-block self-loop matmul (always first)
    #   (g, ci, j, start, stop)   — gather chunk ci of group g into block j
    sched = []
    for b in range(n_batches):
        blocks_here = list(range(b * BLOCKS_PER_BATCH,
                                 min((b + 1) * BLOCKS_PER_BATCH, n_blocks)))
        ci = [0] * N_GROUPS
        chunks = []
        for j in blocks_here:
            tot = int(nch[b, :, j].sum()) + 1       # +1 for the self entry
            chunks.append(("self", j, True, tot == 1))
            seen = 1
            for g in range(N_GROUPS):
                for _ in range(int(nch[b, g, j])):
                    seen += 1
                    chunks.append((g, ci[g], j, False, seen == tot))
                    ci[g] += 1
        sched.append(chunks)
    plan.sched = sched
    plan.call_nch = [[int(plan.nch[b, g].sum()) for g in range(N_GROUPS)]
                     for b in range(n_batches)]
    assert max(c for row in plan.call_nch for c in row) * P <= 9216

    flat_off = np.zeros(counts.size + 1, np.int64)
    np.cumsum(counts.ravel(), out=flat_off[1:])
    starts = flat_off[:-1].reshape(counts.shape)

    rng = np.random.default_rng(12345)
    idx_parts, dl_parts, cv_parts = [], [], []
    for k in range(N_CORES):
        k_idx, k_dl, k_cv = [], [], []
        for b in range(n_batches):
            for g in range(N_GROUPS):
                if plan.call_nch[b][g] == 0:
                    continue
                call_idx, call_dl, call_cv = [], [], []
                for j in range(n_blocks):
                    n_pad = int(nch[b, g, j]) * P
                    if n_pad == 0:
                        continue
                    o = starts[k, b, g, j]
                    cnt = counts[k, b, g, j]
                    si = np.zeros(n_pad, np.int16)
                    dli = np.zeros(n_pad, np.float32)
                    cvi = np.zeros(n_pad, np.float32)
                    si[:cnt] = c_srcloc[o:o + cnt]
                    dli[:cnt] = c_dl[o:o + cnt]
                    cvi[:cnt] = c_c[o:o + cnt]
                    # shuffle inside the cell: kills the sorted-idx gather
                    # penalty; pad rows (c=0) mix in harmlessly
                    perm = rng.permutation(n_pad)
                    call_idx.append(si[perm])
                    call_dl.append(dli[perm])
                    call_cv.append(cvi[perm])
                ci_arr = np.concatenate(call_idx)
                nidx = len(ci_arr)
                wrapped = np.tile(ci_arr.reshape(nidx // 16, 16).T, (8, 1))
                k_idx.append(wrapped.ravel())
                k_dl.append(np.concatenate(call_dl).reshape(-1, P).T.ravel())
                k_cv.append(np.concatenate(call_cv).reshape(-1, P).T.ravel())
        idx_parts.append(np.concatenate(k_idx).astype(np.int16))
        dl_parts.append(np.concatenate(k_dl).astype(np.float32))
        cv_parts.append(np.concatenate(k_cv).astype(np.float32))
    plan.idx = idx_parts
    plan.dl = dl_parts
    plan.cv = cv_parts

    # self-loop coefficients dinv^2 per core, packed [128, n_blocks]
    d2 = np.zeros((N_CORES, n_blocks, P), np.float32)
    dv2 = (dinv * dinv).astype(np.float32)
    for k in range(N_CORES):
        for j in range(n_blocks):
            lo = k * SH + j * P
            hi = min(lo + P, (k + 1) * SH)
            if lo < hi:
                d2[k, j, :hi - lo] = dv2[lo:hi]
    plan.d2_cols = d2.transpose(0, 2, 1).copy()  # [cores, 128, n_blocks]

    # pooling metadata: absolute graph id per node (f32), -1 for pad rows
    bl_cols = np.full((N_CORES, n_blocks, P), -1.0, np.float32)
    for k in range(N_CORES):
        for j in range(n_blocks):
            lo = k * SH + j * P
            hi = min(lo + P, (k + 1) * SH)
            if lo < hi:
                bl_cols[k, j, :hi - lo] = batch[lo:hi].astype(np.float32)
    assert bl_cols.max() < N_GRAPHS
    plan.bl_cols = bl_cols
    return plan


# --------------------------------------------------------------------------
# Device kernel build
# --------------------------------------------------------------------------
def _build_nc(plan):
    N, SH = plan.N, plan.SH
    n_blocks, n_batches = plan.n_blocks, plan.n_batches
    SH_PAD = n_blocks * P
    f32, bf16, i16 = mybir.dt.float32, mybir.dt.bfloat16, mybir.dt.int16
    AF = mybir.ActivationFunctionType
    OP = mybir.AluOpType

    nc = bacc.Bacc(None, target_bir_lowering=False, num_devices=N_CORES,
                   num_swdge_queues=2)

    n_idx16 = plan.idx[0].size // P
    n_ch_tot = plan.dl[0].size // P

    xt = nc.dram_tensor("xt", [N, P], bf16, kind="ExternalInput")
    sxt_d = nc.dram_tensor("sxt", [P, SH_PAD], bf16, kind="ExternalInput")
    idx_d = nc.dram_tensor("idxd", [P * n_idx16], i16, kind="ExternalInput")
    dl_d = nc.dram_tensor("dld", [P * n_ch_tot], f32, kind="ExternalInput")
    cv_d = nc.dram_tensor("cvd", [P * n_ch_tot], f32, kind="ExternalInput")
    w1_d = nc.dram_tensor("w1", [IN_DIM, HID], f32, kind="ExternalInput")
    w2_d = nc.dram_tensor("w2", [HID, HID], f32, kind="ExternalInput")
    wm1_d = nc.dram_tensor("wm1", [HID, HID], f32, kind="ExternalInput")
    wm2_d = nc.dram_tensor("wm2", [HID, OUT_DIM], f32, kind="ExternalInput")
    b1_d = nc.dram_tensor("b1", [HID, 1], f32, kind="ExternalInput")
    b2_d = nc.dram_tensor("b2", [HID, 1], f32, kind="ExternalInput")
    bm1_d = nc.dram_tensor("bm1", [HID, 1], f32, kind="ExternalInput")
    bm2_d = nc.dram_tensor("bm2", [OUT_DIM, 1], f32, kind="ExternalInput")
    bl_d = nc.dram_tensor("bl", [P, n_blocks], f32, kind="ExternalInput")
    d2_d = nc.dram_tensor("d2", [P, n_blocks], f32, kind="ExternalInput")
    out_d = nc.dram_tensor("out", [OUT_DIM, N_GRAPHS], f32, kind="ExternalOutput")

    with tile.TileContext(nc) as tc:
        with (
            tc.tile_pool(name="const", bufs=1) as cpool,
            tc.tile_pool(name="meta", bufs=6) as mpool,
            tc.tile_pool(name="gat", bufs=4) as gpool,
            tc.tile_pool(name="selfh", bufs=1) as hpool,
            tc.tile_pool(name="work", bufs=3) as wpool,
            tc.tile_pool(name="ps", bufs=2, space="PSUM") as ppool,
            tc.tile_pool(name="dram", bufs=1, space="DRAM") as dpool,
        ):
            # ---- constants ----
            iota_f = cpool.tile([P, P], f32)
            nc.gpsimd.iota(iota_f[:], pattern=[[1, P]], base=0, channel_multiplier=0,
                           allow_small_or_imprecise_dtypes=True)
            iota_fb = cpool.tile([P, P], bf16)
            nc.vector.tensor_copy(iota_fb[:], iota_f[:])
            iota_g = cpool.tile([P, N_GRAPHS], f32)
            nc.gpsimd.iota(iota_g[:], pattern=[[1, N_GRAPHS]], base=0,
                           channel_multiplier=0,
                           allow_small_or_imprecise_dtypes=True)
            ident = cpool.tile([P, P], bf16)
            make_identity(nc, ident[:])

            w1b = cpool.tile([IN_DIM, HID], bf16)
            nc.gpsimd.dma_start(w1b[:], w1_d[:])      # SWDGE cast f32->bf16
            w2b = cpool.tile([HID, HID], bf16)
            nc.gpsimd.dma_start(w2b[:], w2_d[:])
            wm1b = cpool.tile([HID, HID], bf16)
            nc.gpsimd.dma_start(wm1b[:], wm1_d[:])
            wm2b = cpool.tile([HID, OUT_DIM], bf16)
            nc.gpsimd.dma_start(wm2b[:], wm2_d[:])
            b1s = cpool.tile([HID, 1], f32)
            nc.sync.dma_start(b1s[:], b1_d[:])
            b2s = cpool.tile([HID, 1], f32)
            nc.sync.dma_start(b2s[:], b2_d[:])
            bm1s = cpool.tile([HID, 1], f32)
            nc.sync.dma_start(bm1s[:], bm1_d[:])
            bm2s = cpool.tile([OUT_DIM, 1], f32)
            nc.sync.dma_start(bm2s[:], bm2_d[:])
            bls = cpool.tile([P, n_blocks], f32)
            nc.sync.dma_start(bls[:], bl_d[:])
            d2s = cpool.tile([P, n_blocks], f32)
            nc.sync.dma_start(d2s[:], d2_d[:])

            # per-block self one-hot (q==p)*dinv2[p] = identity scaled per row
            dself = []
            for j in range(n_blocks):
                dt_ = cpool.tile([P, P], bf16, name=f"dself{j}")
                nc.vector.tensor_scalar(
                    out=dt_[:], in0=ident[:],
                    scalar1=d2s[:, j:j + 1], scalar2=None,
                    op0=OP.mult)
                dself.append(dt_)

            # core's own x shard for layer-1 self term: [128, n_blocks*128]
            selfx = cpool.tile([P, SH_PAD], bf16)
            nc.sync.dma_start(selfx[:], sxt_d[:])

            h1_shard = dpool.tile([SH_PAD, HID], bf16)
            h1_table = dpool.tile([N, HID], bf16, addr_space="Shared")
            cc_in = dpool.tile([P, N_GRAPHS], f32)
            cc_out = dpool.tile([P, N_GRAPHS], f32, addr_space="Shared")

            pool_ps = ppool.tile([HID, N_GRAPHS], f32, tag="pw", bufs=1,
                                 name="pool_ps")

            selfh = []  # layer-1 per-block outputs kept for layer-2 self term

            def layer(lnum, table, feat_dim):
                io = {"idx": 0, "ch": 0}
                qn = {"q": 0}
                for b in range(n_batches):
                    agg = ppool.tile([feat_dim, P * BLOCKS_PER_BATCH], f32,
                                     tag="agg", name=f"agg{lnum}_{b}")
                    gts, dls, cvs = {}, {}, {}
                    for g in range(N_GROUPS):
                        ncall = plan.call_nch[b][g]
                        if ncall == 0:
                            continue
                        nidx = ncall * P
                        s16 = nidx // 16
                        idx_t = mpool.tile([P, s16], i16, tag="idx",
                                           name=f"idx{lnum}_{b}_{g}")
                        nc.sync.dma_start(
                            idx_t[:],
                            idx_d[P * io["idx"]: P * (io["idx"] + s16)]
                            .rearrange("(p c) -> p c", p=P))
                        dl_t = mpool.tile([P, ncall], f32, tag="dl",
                                          name=f"dl{lnum}_{b}_{g}")
                        nc.sync.dma_start(
                            dl_t[:],
                            dl_d[P * io["ch"]: P * (io["ch"] + ncall)]
                            .rearrange("(p c) -> p c", p=P))
                        cv_t = mpool.tile([P, ncall], f32, tag="cv",
                                          name=f"cv{lnum}_{b}_{g}")
                        nc.sync.dma_start(
                            cv_t[:],
                            cv_d[P * io["ch"]: P * (io["ch"] + ncall)]
                            .rearrange("(p c) -> p c", p=P))
                        tab_ap = table[g * plan.grp_size:
                                       min((g + 1) * plan.grp_size, N), :]
                        go = gpool.tile([P, ncall, P], bf16, tag="g",
                                        name=f"g{lnum}_{b}_{g}")
                        nc.gpsimd.dma_gather(
                            out_ap=go[:],
                            in_ap=tab_ap,
                            idxs_ap=idx_t[:],
                            num_idxs=nidx,
                            num_idxs_reg=nidx,
                            elem_size=P,
                            single_packet=False,
                            queue_num=qn["q"] % 2,
                        )
                        qn["q"] += 1
                        gts[g], dls[g], cvs[g] = go, dl_t, cv_t
                        io["idx"] += s16
                        io["ch"] += ncall
                    for ent in plan.sched[b]:
                        if ent[0] == "self":
                            _, j, st, sp = ent
                            jj = j - b * BLOCKS_PER_BATCH
                            if lnum == 1:
                                sl = selfx[:, j * P: j * P + feat_dim]
                            else:
                                sl = selfh[j][:, :feat_dim]
                            nc.tensor.matmul(
                                out=agg[:, jj * P:(jj + 1) * P],
                                lhsT=sl, rhs=dself[j][:],
                                start=st, stop=sp)
                            continue
                        (g, ci, j, st, sp) = ent
                        jj = j - b * BLOCKS_PER_BATCH
                        sel = wpool.tile([P, P], bf16, tag="sel",
                                         name=f"sel{lnum}_{b}_{g}_{ci}")
                        nc.vector.tensor_scalar(
                            out=sel[:], in0=iota_fb[:],
                            scalar1=dls[g][:, ci:ci + 1],
                            scalar2=cvs[g][:, ci:ci + 1],
                            op0=OP.is_equal, op1=OP.mult)
                        gsrc = gts[g][:, ci, :feat_dim]
                        nc.tensor.matmul(
                            out=agg[:, jj * P:(jj + 1) * P],
                            lhsT=gsrc,
                            rhs=sel[:],
                            start=st, stop=sp)
                    # flush the batch
                    for j in range(b * BLOCKS_PER_BATCH,
                                   min((b + 1) * BLOCKS_PER_BATCH, n_blocks)):
                        jj = j - b * BLOCKS_PER_BATCH
                        o_t = wpool.tile([feat_dim, P], bf16, tag="o",
                                         name=f"o{lnum}_{b}_{j}")
                        nc.any.tensor_copy(o_t[:], agg[:, jj * P:(jj + 1) * P])
                        zp = ppool.tile([HID, P], f32, tag="ztr",
                                        name=f"zp{lnum}_{b}_{j}")
                        wmat = w1b if lnum == 1 else w2b
                        bvec = b1s if lnum == 1 else b2s
                        nc.tensor.matmul(out=zp[:], lhsT=wmat[:], rhs=o_t[:],
                                         start=True, stop=True)
                        zs = wpool.tile([HID, P], bf16, tag="zs",
                                        name=f"zs{lnum}_{b}_{j}")
                        nc.scalar.activation(zs[:], zp[:], AF.Relu, bias=bvec[:, :1])
                        trp = ppool.tile([P, HID], bf16, tag="tr",
                                         name=f"trp{lnum}_{b}_{j}")
                        nc.tensor.transpose(out=trp[:], in_=zs[:], identity=ident[:])
                        if lnum == 1:
                            hb = hpool.tile([P, HID], bf16, tag=f"hb{j}",
                                            name=f"hb{j}")
                            selfh.append(hb)
                            nc.any.tensor_copy(hb[:], trp[:])
                            nc.sync.dma_start(h1_shard[j * P:(j + 1) * P, :], hb[:])
                        else:
                            hb = wpool.tile([P, HID], bf16, tag="hb",
                                            name=f"hb2_{b}_{j}")
                            nc.any.tensor_copy(hb[:], trp[:])
                            selB = wpool.tile([P, N_GRAPHS], bf16, tag="selB",
                                              name=f"selB{b}_{j}")
                            nc.vector.tensor_scalar(
                                out=selB[:], in0=iota_g[:],
                                scalar1=bls[:, j:j + 1], scalar2=None,
                                op0=OP.is_equal)
                            nc.tensor.matmul(out=pool_ps[:], lhsT=hb[:],
                                             rhs=selB[:],
                                             start=(j == 0),
                                             stop=(j == n_blocks - 1))

            def early_out():
                outf = cpool.tile([OUT_DIM, N_GRAPHS], f32, name="outf_e")
                nc.vector.memset(outf[:], 0.0)
                nc.sync.dma_start(out_d[:], outf[:])

            # ---- layer 1 (aggregate raw x in 64-dim space) ----
            layer(1, xt, IN_DIM)
            done = _STOP == "l1"

            # ---- AllGather h1 ----
            if not done:
                nc.gpsimd.collective_compute(
                    "AllGather", mybir.AluOpType.bypass,
                    replica_groups=[list(range(N_CORES))],
                    ins=[h1_shard[0:SH, :].opt()],
                    outs=[h1_table[:].opt()],
                )
                done = _STOP == "ag"

            # ---- layer 2 ----
            if not done:
                layer(2, h1_table, HID)
                done = _STOP == "l2"

            # ---- pooled AllReduce + MLP head ----
            if done:
                early_out()
                do_tail = False
            else:
                do_tail = True
            if do_tail:
                pooledT = cpool.tile([P, N_GRAPHS], f32)
                nc.any.tensor_copy(pooledT[:], pool_ps[:])
                nc.sync.dma_start(cc_in[:], pooledT[:])
                nc.gpsimd.collective_compute(
                    "AllReduce", mybir.AluOpType.add,
                    replica_groups=[list(range(N_CORES))],
                    ins=[cc_in[:].opt()],
                    outs=[cc_out[:].opt()],
                )
                pall = cpool.tile([P, N_GRAPHS], f32)
                nc.sync.dma_start(pall[:], cc_out[:])
                pbf = cpool.tile([P, N_GRAPHS], bf16)
                nc.vector.tensor_copy(pbf[:], pall[:])
                m1p = ppool.tile([HID, N_GRAPHS], f32, tag="agg", name="m1p")
                nc.tensor.matmul(out=m1p[:], lhsT=wm1b[:], rhs=pbf[:],
                                 start=True, stop=True)
                m1s = cpool.tile([HID, N_GRAPHS], bf16)
                nc.scalar.activation(m1s[:], m1p[:], AF.Relu, bias=bm1s[:, :1])
                m2p = ppool.tile([OUT_DIM, N_GRAPHS], f32, tag="ztr", name="m2p")
                nc.tensor.matmul(out=m2p[:], lhsT=wm2b[:], rhs=m1s[:],
                                 start=True, stop=True)
                outf = cpool.tile([OUT_DIM, N_GRAPHS], f32)
                nc.vector.tensor_scalar(out=outf[:], in0=m2p[:],
                                        scalar1=bm2s[:, :1], scalar2=None,
                                        op0=OP.add)
                nc.sync.dma_start(out_d[:], outf[:])

    nc.finalize()
    return nc


# --------------------------------------------------------------------------
# Public entry point
# --------------------------------------------------------------------------
def kernel(x, edge_index, batch, edge_attr, W1, b1, W2, b2, Wm1, bm1, Wm2, bm2):
    x = np.asarray(x, np.float32)
    edge_index = np.asarray(edge_index, np.int64)
    batch_np = np.asarray(batch, np.int64)
    edge_attr = np.asarray(edge_attr, np.float32)
    N = x.shape[0]

    _install_profhook()
    plan = _build_plan(x, edge_index, batch_np, edge_attr)
    SH, SH_PAD = plan.SH, plan.n_blocks * P

    # padded bf16 x table [N, 128] (first 64 cols = x)
    xt = np.zeros((N, P), BF)
    xt[:, :IN_DIM] = x.astype(BF)

    in_maps = []
    for k in range(N_CORES):
        # per-core own shard, [128, SH_PAD] bf16: sxt[p, j*128+f] = x[k*SH+j*128+p, f]
        sxt = np.zeros((plan.n_blocks, P, P), np.float32)
        lo, hi = k * SH, (k + 1) * SH
        shard = np.zeros((SH_PAD, P), np.float32)
        shard[:SH, :IN_DIM] = x[lo:hi]
        sxt = shard.reshape(plan.n_blocks, P, P).transpose(1, 0, 2).reshape(P, SH_PAD)
        in_maps.append({
            "xt": xt,
            "sxt": sxt.astype(BF),
            "idxd": plan.idx[k],
            "dld": plan.dl[k],
            "cvd": plan.cv[k],
            "w1": np.asarray(W1, np.float32),
            "w2": np.asarray(W2, np.float32),
            "wm1": np.asarray(Wm1, np.float32),
            "wm2": np.asarray(Wm2, np.float32),
            "b1": np.asarray(b1, np.float32).reshape(HID, 1),
            "b2": np.asarray(b2, np.float32).reshape(HID, 1),
            "bm1": np.asarray(bm1, np.float32).reshape(HID, 1),
            "bm2": np.asarray(bm2, np.float32).reshape(OUT_DIM, 1),
            "bl": plan.bl_cols[k].T.copy(),     # [128, n_blocks]
            "d2": plan.d2_cols[k].copy(),       # [128, n_blocks]
        })

    nc = _build_nc(plan)
    res = run_bass_kernel_spmd(nc, in_maps, list(range(N_CORES)), trace=_TRACE)
    if _TRACE:
        kernel.last_exec_time_ns = res.exec_time_ns
        kernel.last_results = res
    out = np.asarray(res.results[0]["out"], np.float32)  # [10, 512]
    return np.ascontiguousarray(out.T)
